# revision 1
# baseline (speedup 1.0000x reference)
"""Trainium2 Bass kernel for the EEG SNN model (LIF -> LSNN -> LIF classifier).

Data-parallel over 8 NeuronCores: batch 64 -> 8 per core. The three
sequential T=8192 scans are parallelized with a chunked multi-pass scheme
(chunks in the free axis; carry influence decays below f32 resolution
within a chunk, validated bitwise against the reference semantics).
"""
import numpy as np

import concourse.bass as bass
import concourse.bacc as bacc
import concourse.mybir as mybir
from concourse import tile
from concourse.bass_utils import run_bass_kernel_spmd

F32 = mybir.dt.float32
OP = mybir.AluOpType
ACTF = mybir.ActivationFunctionType

# model constants
VTH = 0.2
TH2 = float(np.float32(0.2) / np.float32(0.9))  # threshold for t = v_dec/0.9
B = 64          # global batch
BC = 8          # batch per core
NCORES = 8
C = 64          # eeg channels
H = 10          # hidden
O = 2           # outputs
T = 8192

NGRP = 1                        # LSNN chunk-group interleave factor
Z_ENG = "dve"                   # engine for the spike compare (dve|pool)
VN_ENG = "dve"                  # engine for the VT mask op (dve|pool)
# chunking config
L1, N1 = 16, 512                # LIF1: 512 chunks x 16 steps, 2 passes
L2, N2, NPASS2 = 256, 32, 3     # LSNN: 32 chunks x 256 steps, 3 passes
HEAL2 = 128                     # final pass re-runs only this prefix
L3, N3 = 32, 32                 # LIF2: 128 lanes, 32 chunks x 32 steps, 2 passes
TT = 512                        # t-tile for matmul phases
NTT = T // TT                   # 16
TL = T // 8                     # 1024 t per LIF2 lane


def emit_program(nc):
    eeg_d = nc.declare_dram_parameter("eeg", [BC, C, T], F32, isOutput=False)
    wpack_d = nc.declare_dram_parameter("wpack", [128, 593], F32, isOutput=False)
    out_d = nc.declare_dram_parameter("out", [16, 1], F32, isOutput=True)

    with tile.TileContext(nc) as tc:
        _emit(nc, tc, eeg_d, wpack_d, out_d)
    return nc


def _emit(nc, tc, eeg_d, wpack_d, out_d):
    with (
        tc.tile_pool(name="singles", bufs=1) as singles,
        tc.tile_pool(name="eegp", bufs=6) as eegp,
        tc.tile_pool(name="state", bufs=3) as state,
        tc.tile_pool(name="small", bufs=3) as small,
        tc.tile_pool(name="psA", bufs=3, space="PSUM") as psA,
        tc.tile_pool(name="psB", bufs=2, space="PSUM") as psB,
    ):
        # ---- persistent SBUF buffers ----
        inp = singles.tile([80, T], F32, tag="big_a")  # front currents
        U1 = singles.tile([80, T], F32, tag="big_b")   # LIF1 membrane
        XI = singles.tile([80, T], F32)     # spikes @ w_in.T, step-major
        Z = singles.tile([96, T], F32)      # LSNN spikes {0,1}; row 80 = ones
        Q16 = None  # allocated later, reuses inp slot
        Q = singles.tile([128, TL], F32)    # classifier currents, packed
        U3 = singles.tile([128, TL], F32)   # LIF2 membrane, packed

        WP = singles.tile([128, 593], F32)
        nc.sync.dma_start(WP[:], wpack_d.ap())
        wf = WP[:, 0:320]
        bias80 = WP[0:80, 320:321]
        win = WP[0:80, 321:401]
        wrec = WP[0:80, 401:481]
        eye72 = WP[0:80, 481:561]
        wcls = WP[0:81, 561:577]
        ones_sum = WP[:, 577:593]

        # ones row for the classifier bias trick lives at partition 80
        nc.vector.memset(Z[64:96, :], 1.0)

        # PE warmup: consume the whole weight tile once so later matmuls
        # never need a DMA wait for weights (PE ISA allows 1 sem wait/matmul)
        wps = psA.tile([128, 465], F32, tag="mmps")
        nc.tensor.matmul(wps[:], WP[:, 0:128], WP[:, 128:593],
                         start=True, stop=True)

        # ================= FRONT: inp = w_front @ eeg + b_front ============
        eeg_ap = eeg_d.ap()
        for tt in range(NTT):
            ps = psA.tile([80, TT], F32, tag="mmps")
            for pair in range(BC // 2):
                et = eegp.tile([128, TT], F32, tag="eeg")
                src = eeg_ap[2 * pair:2 * pair + 2, :, tt * TT:(tt + 1) * TT]
                nc.sync.dma_start(et[:], src.rearrange("a c t -> (a c) t"))
                nc.tensor.matmul(ps[:], wf[:, 80 * pair:80 * (pair + 1)],
                                 et[:], start=(pair == 0), stop=(pair == 3))
            dst = inp[:, tt * TT:(tt + 1) * TT]
            nc.vector.tensor_scalar(out=dst, in0=ps[:], scalar1=bias80,
                                    scalar2=None, op0=OP.add)

        # ================= LIF1: chunked 2-pass scan =======================
        Xv = inp[:].rearrange("p (c s) -> p c s", s=L1)
        Uv = U1[:].rearrange("p (c s) -> p c s", s=L1)

        # two interleaved half-streams: the independent neighbor op fills
        # each same-engine RAW drain gap (values identical, disjoint halves)
        NHF = N1 // 2
        prevs = []
        for h in range(2):
            u = state.tile([80, NHF], F32, tag=f"u1{h}")
            nc.vector.memset(u[:], 0.0)
            prevs.append(u[:])
        for s in range(L1):  # pass 1
            gs = []
            for h in range(2):
                g = state.tile([80, NHF], F32, tag=f"g1{h}")
                nc.vector.scalar_tensor_tensor(out=g[:], in0=prevs[h],
                                               scalar=VTH, in1=prevs[h],
                                               op0=OP.is_le, op1=OP.mult)
                gs.append(g)
            for h in range(2):
                un = state.tile([80, NHF], F32, tag=f"u1{h}")
                nc.vector.scalar_tensor_tensor(out=un[:], in0=gs[h][:],
                                               scalar=0.25,
                                               in1=Xv[:, h * NHF:(h + 1) * NHF, s],
                                               op0=OP.mult, op1=OP.add)
                prevs[h] = un[:]
        uis = []
        for h in range(2):
            ui = state.tile([80, NHF], F32, tag=f"u1i{h}")
            if h == 0:
                nc.vector.memset(ui[:, 0:1], 0.0)
            else:
                nc.vector.tensor_copy(ui[:, 0:1], prevs[0][:, NHF - 1:NHF])
            nc.vector.tensor_copy(ui[:, 1:NHF], prevs[h][:, 0:NHF - 1])
            uis.append(ui[:])
        prevs = uis
        for s in range(L1):  # pass 2 -> writes U1
            gs = []
            for h in range(2):
                g = state.tile([80, NHF], F32, tag=f"g1{h}")
                nc.vector.scalar_tensor_tensor(out=g[:], in0=prevs[h],
                                               scalar=VTH, in1=prevs[h],
                                               op0=OP.is_le, op1=OP.mult)
                gs.append(g)
            for h in range(2):
                nc.vector.scalar_tensor_tensor(
                    out=Uv[:, h * NHF:(h + 1) * NHF, s], in0=gs[h][:],
                    scalar=0.25, in1=Xv[:, h * NHF:(h + 1) * NHF, s],
                    op0=OP.mult, op1=OP.add)
                prevs[h] = Uv[:, h * NHF:(h + 1) * NHF, s]

        # ====== spikes S1 + XI = S1 @ w_in.T (written step-major) ==========
        XIcs = XI[:].rearrange("p (s c) -> p c s", c=N2)
        for tt in range(NTT):
            s1 = eegp.tile([80, TT], F32, tag="s1")
            nc.vector.tensor_scalar(out=s1[:], in0=U1[:, tt * TT:(tt + 1) * TT],
                                    scalar1=VTH, scalar2=None, op0=OP.is_gt)
            ps = psA.tile([80, TT], F32, tag="mmps")
            nc.tensor.matmul(ps[:], win, s1[:], start=True, stop=True)
            nc.vector.tensor_copy(XIcs[:, 2 * tt:2 * tt + 2, :], ps[:])

        # ============ LSNN v3: one constant-stationary matmul per step =====
        # States: z (spikes), VT = (1-z)*T with T = 10*v_dec; the synaptic
        # current i lives folded inside the PSUM accumulator P = i_jump:
        #   K_{s+1} = 0.8*P_s + xin_{s+1}   (DVE, PSUM->PSUM)
        #   P_{s+1} = K_{s+1} + W_rec.z_s   (PE accumulate, stationary fixed)
        #   T_s     = 0.9*VT_{s-1} + P_s ; z_s = T_s>2 ; VT_s = (T_s<=2)*T_s
        # Two independent chunk-groups interleave to hide chain latency.
        XIsc = XI[:].rearrange("p (s c) -> p s c", c=N2)
        Zsc = Z[0:80, :].rearrange("p (s c) -> p s c", c=N2)
        NH = N2 // NGRP
        TH10 = 2.0

        st = {}
        for grp in range(NGRP):
            c0 = grp * NH
            z = state.tile([80, NH], F32, tag=f"z2{grp}")
            vt = state.tile([80, NH], F32, tag=f"v2{grp}")
            nc.vector.memset(z[:], 0.0)
            nc.vector.memset(vt[:], 0.0)
            p0 = psB.tile([80, NH], F32, tag=f"lps{grp}")
            nc.vector.tensor_copy(p0[:], XIsc[:, 0, c0:c0 + NH])
            st[grp] = (z[:], vt[:], p0)

        for p in range(NPASS2):
            final = p == NPASS2 - 1
            write_z = p >= 1
            nsteps = HEAL2 if final else L2
            if p > 0:
                ends = st
                st = {}
                for grp in range(NGRP):
                    c0 = grp * NH
                    zi = state.tile([80, NH], F32, tag=f"z2i{grp}")
                    vi = state.tile([80, NH], F32, tag=f"v2i{grp}")
                    ii = state.tile([80, NH], F32, tag=f"i2i{grp}")
                    for t_, e_, eprev_ in (
                        (zi, ends[grp][0], ends[0][0]),
                        (vi, ends[grp][1], ends[0][1]),
                        (ii, ends[grp][2], ends[0][2]),
                    ):
                        if grp == 0:
                            nc.vector.memset(t_[:, 0:1], 0.0)
                        else:
                            nc.vector.tensor_copy(t_[:, 0:1],
                                                  eprev_[:, NH - 1:NH])
                        nc.vector.tensor_copy(t_[:, 1:NH], e_[:, 0:NH - 1])
                    p0 = psB.tile([80, NH], F32, tag=f"lps{grp}")
                    nc.vector.scalar_tensor_tensor(
                        out=p0[:], in0=ii[:], scalar=0.0,
                        in1=XIsc[:, 0, c0:c0 + NH], op0=OP.bypass, op1=OP.add)
                    nc.tensor.matmul(p0[:], wrec, zi[:], start=False,
                                     stop=True, skip_group_check=True)
                    st[grp] = (zi[:], vi[:], p0)
            for s in range(nsteps):
                for grp in range(NGRP):
                    z_prev, vt_prev, pcur = st[grp]
                    c0 = grp * NH
                    tn = state.tile([80, NH], F32, tag=f"t2{grp}")
                    nc.vector.scalar_tensor_tensor(out=tn[:], in0=vt_prev,
                                                   scalar=0.9, in1=pcur[:],
                                                   op0=OP.mult, op1=OP.add)
                    # opA right after tn: independent of tn, fills the
                    # same-engine RAW drain gap before the z compare
                    pnxt = None
                    if s < nsteps - 1:
                        pnxt = psB.tile([80, NH], F32, tag=f"lps{grp}")
                        nc.vector.scalar_tensor_tensor(
                            out=pnxt[:], in0=pcur[:], scalar=0.8,
                            in1=XIsc[:, s + 1, c0:c0 + NH],
                            op0=OP.mult, op1=OP.add)
                    if write_z:
                        z_dst = Zsc[:, s, c0:c0 + NH]
                    else:
                        zt = state.tile([80, NH], F32, tag=f"z2{grp}")
                        z_dst = zt[:]
                    z_e = nc.vector if Z_ENG == "dve" else nc.gpsimd
                    z_e.tensor_scalar(out=z_dst, in0=tn[:],
                                      scalar1=TH10, scalar2=None,
                                      op0=OP.is_gt)
                    vn = state.tile([80, NH], F32, tag=f"v2{grp}")
                    v_e = nc.vector if VN_ENG == "dve" else nc.gpsimd
                    v_e.scalar_tensor_tensor(out=vn[:], in0=tn[:],
                                             scalar=TH10, in1=tn[:],
                                             op0=OP.is_le, op1=OP.mult)
                    if s < nsteps - 1:
                        nc.tensor.matmul(pnxt[:], wrec, z_dst, start=False,
                                         stop=True, skip_group_check=True)
                        st[grp] = (z_dst, vn[:], pnxt)
                    elif not final:
                        ie = state.tile([80, NH], F32, tag=f"ie{grp}")
                        nc.vector.tensor_scalar(out=ie[:], in0=pcur[:],
                                                scalar1=0.8, scalar2=None,
                                                op0=OP.mult)
                        st[grp] = (z_dst, vn[:], ie[:])

        # ========== classifier: q = w_cls @ z + b_cls ======================
        Q16 = singles.tile([16, T], F32, tag="big_a")
        for tt in range(NTT):
            ps = psA.tile([16, TT], F32, tag="mmps")
            Zcs = Z[0:81, :].rearrange("p (s c) -> p c s", c=N2)
            nc.tensor.matmul(ps[:], wcls, Zcs[:, 2 * tt:2 * tt + 2, :],
                             start=True, stop=True)
            dst = Q16[:, tt * TT:(tt + 1) * TT]
            nc.vector.tensor_copy(dst, ps[:])
        # repack [16, 8192] -> [128, 1024]: lane p = 16*g + (b*2+o)
        for g in range(8):
            nc.sync.dma_start(Q[16 * g:16 * (g + 1), :],
                              Q16[:, TL * g:TL * (g + 1)])

        # ================= LIF2: chunked 2-pass scan (128 lanes) ===========
        Qv = Q[:].rearrange("p (c s) -> p c s", s=L3)
        U3v = U3[:].rearrange("p (c s) -> p c s", s=L3)

        NH3 = N3 // 2
        prevs3 = []
        for h in range(2):
            u3 = state.tile([128, NH3], F32, tag=f"u3{h}")
            nc.vector.memset(u3[:], 0.0)
            prevs3.append(u3[:])
        for s in range(L3):
            gs = []
            for h in range(2):
                g = state.tile([128, NH3], F32, tag=f"g3{h}")
                nc.vector.scalar_tensor_tensor(out=g[:], in0=prevs3[h],
                                               scalar=VTH, in1=prevs3[h],
                                               op0=OP.is_le, op1=OP.mult)
                gs.append(g)
            for h in range(2):
                un = state.tile([128, NH3], F32, tag=f"u3{h}")
                nc.vector.scalar_tensor_tensor(
                    out=un[:], in0=gs[h][:], scalar=0.25,
                    in1=Qv[:, h * NH3:(h + 1) * NH3, s],
                    op0=OP.mult, op1=OP.add)
                prevs3[h] = un[:]
        uis3 = []
        for h in range(2):
            ui3 = state.tile([128, NH3], F32, tag=f"u3i{h}")
            if h == 0:
                nc.vector.memset(ui3[:, 0:1], 0.0)
                # carry across partition groups: lane p <- lane p-16 last chunk
                nc.sync.dma_start(ui3[16:128, 0:1],
                                  prevs3[1][0:112, NH3 - 1:NH3])
            else:
                nc.vector.tensor_copy(ui3[:, 0:1], prevs3[0][:, NH3 - 1:NH3])
            nc.vector.tensor_copy(ui3[:, 1:NH3], prevs3[h][:, 0:NH3 - 1])
            uis3.append(ui3[:])
        prevs3 = uis3
        for s in range(L3):
            gs = []
            for h in range(2):
                g = state.tile([128, NH3], F32, tag=f"g3{h}")
                nc.vector.scalar_tensor_tensor(out=g[:], in0=prevs3[h],
                                               scalar=VTH, in1=prevs3[h],
                                               op0=OP.is_le, op1=OP.mult)
                gs.append(g)
            for h in range(2):
                nc.vector.scalar_tensor_tensor(
                    out=U3v[:, h * NH3:(h + 1) * NH3, s], in0=gs[h][:],
                    scalar=0.25, in1=Qv[:, h * NH3:(h + 1) * NH3, s],
                    op0=OP.mult, op1=OP.add)
                prevs3[h] = U3v[:, h * NH3:(h + 1) * NH3, s]

        # ================= spike count + mean ==============================
        sp = singles.tile([128, TL], F32, tag="big_b")
        nc.vector.tensor_scalar(out=sp[:], in0=U3[:], scalar1=VTH,
                                scalar2=None, op0=OP.is_gt)
        red = small.tile([128, 1], F32, tag="red")
        nc.vector.tensor_reduce(out=red[:], in_=sp[:], axis=mybir.AxisListType.X,
                                op=OP.add)
        pso = psB.tile([16, 1], F32, tag="lps0")
        nc.tensor.matmul(pso[:], ones_sum, red[:], start=True, stop=True)
        res = small.tile([16, 1], F32, tag="res")
        nc.scalar.activation(res[:], pso[:], ACTF.Copy, scale=1.0 / T)
        nc.sync.dma_start(out_d.ap(), res[:])


_NC_CACHE = None


def _get_program():
    global _NC_CACHE
    if _NC_CACHE is None:
        nc = bacc.Bacc("TRN2", target_bir_lowering=False, debug=False)
        emit_program(nc)
        nc.compile()
        _NC_CACHE = nc
    return _NC_CACHE


def make_in_maps(x, w_front, b_front, w_in, w_rec, w_cls, b_cls):
    x = np.asarray(x, np.float32)
    w_front = np.asarray(w_front, np.float32)
    b_front = np.asarray(b_front, np.float32)
    w_in = np.asarray(w_in, np.float32)
    w_rec = np.asarray(w_rec, np.float32)
    w_cls = np.asarray(w_cls, np.float32)
    b_cls = np.asarray(b_cls, np.float32)

    eeg = np.ascontiguousarray(x[:, 0, 1:-1, :])  # [B, C, T]

    wpack = np.zeros((128, 593), np.float32)
    for pair in range(4):
        for b2 in range(2):
            c0 = pair * 80 + pair * 20 + b2 * 10
            wpack[b2 * 64:(b2 + 1) * 64, c0:c0 + 10] = w_front.T
    wpack[0:80, 320] = np.tile(b_front, 8)
    for b in range(8):
        r = slice(b * 10, (b + 1) * 10)
        wpack[r, 321 + b * 10:321 + (b + 1) * 10] = w_in.T
        wpack[r, 401 + b * 10:401 + (b + 1) * 10] = w_rec.T
        wpack[r, 561 + b * 2:561 + (b + 1) * 2] = w_cls.T
        wpack[80, 561 + b * 2:561 + (b + 1) * 2] = b_cls
    wpack[0:80, 481:561] = 7.2 * np.eye(80)
    for p in range(128):
        wpack[p, 577 + p % 16] = 1.0

    in_maps = []
    for c in range(NCORES):
        in_maps.append({
            "eeg": np.ascontiguousarray(eeg[c * BC:(c + 1) * BC]),
            "wpack": wpack,
        })
    return in_maps


def run_cores(in_maps, **kw):
    nc = _get_program()
    return run_bass_kernel_spmd(nc, in_maps, list(range(NCORES)), **kw)


def kernel(x, w_front, b_front, w_in, w_rec, w_cls, b_cls):
    in_maps = make_in_maps(x, w_front, b_front, w_in, w_rec, w_cls, b_cls)
    res = run_cores(in_maps)
    outs = [res.results[c]["out"].reshape(BC, O) for c in range(NCORES)]
    return np.concatenate(outs, axis=0).astype(np.float32)



# revision 11
# speedup vs baseline: 1.2459x; 1.2459x over previous
"""Trainium2 Bass kernel for the EEG SNN model (LIF -> LSNN -> LIF classifier).

Data-parallel over 8 NeuronCores: batch 64 -> 8 per core. The three
sequential T=8192 scans use chunked multi-pass schemes (carry influence
decays within a chunk; validated against the reference semantics).

LSNN inner loop (the dominant cost) uses an input-current decomposition
  i_jump_s = IX_s + w_rec @ C_s,   C_s = 0.8*C_{s-1} + z_{s-1}
where IX (the x-driven current) is precomputed in bulk with the hardware
scan instruction, and C (spike EMA) lives in SBUF maintained by gpsimd.
This keeps the per-step critical chain to: PE matmul -> DVE tn -> DVE z
-> PE matmul, with no PSUM readback op on the DVE queue.
"""
import numpy as np

import concourse.bass as bass
import concourse.bacc as bacc
import concourse.mybir as mybir
from concourse import tile
from concourse.bass_utils import run_bass_kernel_spmd

F32 = mybir.dt.float32
OP = mybir.AluOpType
ACTF = mybir.ActivationFunctionType

# model constants
VTH = 0.2
B = 64          # global batch
BC = 8          # batch per core
NCORES = 8
C = 64          # eeg channels
H = 10          # hidden
O = 2           # outputs
T = 8192

# chunking config
L1, N1 = 16, 512                # LIF1: 512 chunks x 16 steps, 2 passes
L2 = 128                        # LSNN chunk length
N2 = T // L2                    # 64 chunks
NPASS2 = 4                      # full passes (512 sequential steps)
L3 = 16                         # LIF2 chunk length
TL = T // 8                     # 1024 t per LIF2 lane
N3 = TL // L3                   # 64 chunks per lane
TT = 512                        # t-tile for matmul phases
NTT = T // TT                   # 16
TH10 = 2.0                      # threshold in the x10 scale (v_dec*10 > 2)


def emit_program(nc):
    eeg_d = nc.declare_dram_parameter("eeg", [BC, C, T], F32, isOutput=False)
    wpack_d = nc.declare_dram_parameter("wpack", [128, 593], F32, isOutput=False)
    out_d = nc.declare_dram_parameter("out", [16, 1], F32, isOutput=True)

    with tile.TileContext(nc) as tc:
        _emit(nc, tc, eeg_d, wpack_d, out_d)
    return nc


def _emit(nc, tc, eeg_d, wpack_d, out_d):
    with (
        tc.tile_pool(name="singles", bufs=1) as singles,
        tc.tile_pool(name="eegp", bufs=8) as eegp,
        tc.tile_pool(name="state", bufs=3) as state,
        tc.tile_pool(name="small", bufs=3) as small,
        tc.tile_pool(name="psA", bufs=3, space="PSUM") as psA,
        tc.tile_pool(name="psB", bufs=3, space="PSUM") as psB,
    ):
        # ---- persistent SBUF buffers ----
        inp = singles.tile([80, T], F32, tag="big_a")  # front currents
        U1 = singles.tile([80, T], F32, tag="big_b")   # LIF1 membrane
        XI = singles.tile([80, T], F32)     # spikes @ w_in.T, natural t order
        Z = singles.tile([96, T], F32)      # LSNN spikes; layout col = s*N2+c
        Q = singles.tile([128, TL], F32)    # classifier currents, packed
        U3 = singles.tile([128, TL], F32)   # LIF2 membrane, packed

        WP = singles.tile([128, 593], F32)
        nc.sync.dma_start(WP[:], wpack_d.ap())
        wf = WP[:, 0:320]
        bias80 = WP[0:80, 320:321]
        win = WP[0:80, 321:401]
        wrec = WP[0:80, 401:481]
        wcls = WP[0:81, 561:577]
        ones_sum = WP[:, 577:593]

        # ones row for the classifier bias trick lives at partition 80
        nc.vector.memset(Z[64:96, :], 1.0)
        c08 = singles.tile([80, TT], F32)
        nc.vector.memset(c08[:], 0.8)

        # PE warmup: consume the whole weight tile once so later matmuls
        # never need a DMA wait for weights (PE ISA allows 1 sem wait/matmul)
        wps = psA.tile([128, 465], F32, tag="mmps")
        nc.tensor.matmul(wps[:], WP[:, 0:128], WP[:, 128:593],
                         start=True, stop=True)

        # ================= FRONT: inp = w_front @ eeg + b_front ============
        eeg_ap = eeg_d.ap()
        for tt in range(NTT):
            ps = psA.tile([80, TT], F32, tag="mmps")
            for pair in range(BC // 2):
                et = eegp.tile([128, TT], F32, tag="eeg")
                src = eeg_ap[2 * pair:2 * pair + 2, :, tt * TT:(tt + 1) * TT]
                nc.sync.dma_start(et[:], src.rearrange("a c t -> (a c) t"))
                nc.tensor.matmul(ps[:], wf[:, 80 * pair:80 * (pair + 1)],
                                 et[:], start=(pair == 0), stop=(pair == 3))
            dst = inp[:, tt * TT:(tt + 1) * TT]
            nc.vector.tensor_scalar(out=dst, in0=ps[:], scalar1=bias80,
                                    scalar2=None, op0=OP.add)

        # ================= LIF1: chunked 2-pass scan =======================
        # two interleaved half-streams on DVE: the independent neighbor op
        # fills each same-engine RAW drain gap
        Xv = inp[:].rearrange("p (c s) -> p c s", s=L1)
        Uv = U1[:].rearrange("p (c s) -> p c s", s=L1)
        NHF = N1 // 2

        prevs = []
        for h in range(2):
            u = state.tile([80, NHF], F32, tag=f"u1{h}")
            nc.vector.memset(u[:], 0.0)
            prevs.append(u[:])
        for s in range(L1):  # pass 1
            gs = []
            for h in range(2):
                g = state.tile([80, NHF], F32, tag=f"g1{h}")
                nc.vector.scalar_tensor_tensor(out=g[:], in0=prevs[h],
                                               scalar=VTH, in1=prevs[h],
                                               op0=OP.is_le, op1=OP.mult)
                gs.append(g)
            for h in range(2):
                un = state.tile([80, NHF], F32, tag=f"u1{h}")
                nc.vector.scalar_tensor_tensor(out=un[:], in0=gs[h][:],
                                               scalar=0.25,
                                               in1=Xv[:, h * NHF:(h + 1) * NHF, s],
                                               op0=OP.mult, op1=OP.add)
                prevs[h] = un[:]
        uis = []
        for h in range(2):
            ui = state.tile([80, NHF], F32, tag=f"u1i{h}")
            if h == 0:
                nc.vector.memset(ui[:, 0:1], 0.0)
            else:
                nc.vector.tensor_copy(ui[:, 0:1], prevs[0][:, NHF - 1:NHF])
            nc.vector.tensor_copy(ui[:, 1:NHF], prevs[h][:, 0:NHF - 1])
            uis.append(ui[:])
        prevs = uis
        for s in range(L1):  # pass 2 -> writes U1
            gs = []
            for h in range(2):
                g = state.tile([80, NHF], F32, tag=f"g1{h}")
                nc.vector.scalar_tensor_tensor(out=g[:], in0=prevs[h],
                                               scalar=VTH, in1=prevs[h],
                                               op0=OP.is_le, op1=OP.mult)
                gs.append(g)
            for h in range(2):
                nc.vector.scalar_tensor_tensor(
                    out=Uv[:, h * NHF:(h + 1) * NHF, s], in0=gs[h][:],
                    scalar=0.25, in1=Xv[:, h * NHF:(h + 1) * NHF, s],
                    op0=OP.mult, op1=OP.add)
                prevs[h] = Uv[:, h * NHF:(h + 1) * NHF, s]

        # ====== spikes S1; XI = S1 @ w_in.T (natural t); IX = scan(XI) =====
        # IX reuses inp's memory (inp is dead after LIF1 pass 2)
        IX = singles.tile([80, T], F32, tag="big_a")
        for tt in range(NTT):
            s1 = eegp.tile([80, TT], F32, tag="s1")
            nc.vector.tensor_scalar(out=s1[:], in0=U1[:, tt * TT:(tt + 1) * TT],
                                    scalar1=VTH, scalar2=None, op0=OP.is_gt)
            ps = psA.tile([80, TT], F32, tag="mmps")
            nc.tensor.matmul(ps[:], win, s1[:], start=True, stop=True)
            xisl = XI[:, tt * TT:(tt + 1) * TT]
            nc.scalar.activation(xisl, ps[:], ACTF.Copy, scale=1.0)
            init = 0.0 if tt == 0 else IX[:, tt * TT - 1:tt * TT]
            nc.vector.tensor_tensor_scan(
                out=IX[:, tt * TT:(tt + 1) * TT], data0=c08[:], data1=xisl,
                initial=init, op0=OP.mult, op1=OP.add)

        # ===================== LSNN: chunked NPASS2-pass ===================
        # per step s: bank_s = Act(IX_s) + wrec@C08_{s-1} + wrec@z_{s-1};
        # tn = 0.9*vt + bank; z = tn>2; vt = (tn<=2)*tn   (all x10 scale)
        IXs = IX[:].rearrange("p (c s) -> p s c", s=L2)  # [80, L2, N2]
        Zn = Z[0:80, :]

        c08n = singles.tile([80, N2], F32)
        nc.vector.memset(c08n[:], 0.8)

        gamma = None    # carried C_L tile from previous pass
        vt_end = None   # carried vt at chunk end
        for p in range(NPASS2):
            write_z = p == NPASS2 - 1
            # --- boundary prep: shifted carries ---
            C0 = state.tile([80, N2], F32, tag="c0")
            vt = state.tile([80, N2], F32, tag="vt0")
            if p == 0:
                nc.vector.memset(C0[:], 0.0)
                nc.vector.memset(vt[:], 0.0)
            else:
                nc.vector.memset(C0[:, 0:1], 0.0)
                nc.vector.tensor_copy(C0[:, 1:N2], gamma[:, 0:N2 - 1])
                nc.vector.memset(vt[:, 0:1], 0.0)
                nc.vector.tensor_copy(vt[:, 1:N2], vt_end[:, 0:N2 - 1])
            vt_prev = vt[:]
            zeta_prev = C0[:]          # C_0 = zeta entering step 0
            t1p = state.tile([80, N2], F32, tag="t1")
            nc.gpsimd.tensor_mul(t1p[:], C0[:], c08n[:])
            t1_prev = t1p[:]           # 0.8*C_0 (mmE operand for step 1)

            # step 0 bank: Act init + one matmul with C_0
            bank = psB.tile([80, N2], F32, tag="bank")
            nc.scalar.activation(bank[:], IXs[:, 0, :], ACTF.Copy, scale=1.0)
            nc.tensor.matmul(bank[:], wrec, C0[:], start=False, stop=True,
                             skip_group_check=True)
            # Act-init next bank early
            bank_n = psB.tile([80, N2], F32, tag="bank")
            nc.scalar.activation(bank_n[:], IXs[:, 1, :], ACTF.Copy, scale=1.0)

            for s in range(L2):
                # tn/z/vn on DVE (z is the only op on the critical chain
                # between the two PE matmuls)
                tn = state.tile([80, N2], F32, tag="t2")
                nc.vector.scalar_tensor_tensor(out=tn[:], in0=vt_prev,
                                               scalar=0.9, in1=bank[:],
                                               op0=OP.mult, op1=OP.add)
                if write_z:
                    z_dst = Zn[:, s * N2:(s + 1) * N2]
                else:
                    zt = state.tile([80, N2], F32, tag="z2")
                    z_dst = zt[:]
                nc.vector.tensor_scalar(out=z_dst, in0=tn[:],
                                        scalar1=TH10, scalar2=None,
                                        op0=OP.is_gt)
                vn = state.tile([80, N2], F32, tag="v2")
                nc.vector.scalar_tensor_tensor(out=vn[:], in0=tn[:],
                                               scalar=TH10, in1=tn[:],
                                               op0=OP.is_le, op1=OP.mult)

                # Pool: advance the spike EMA zeta_{s+1} = 0.8*zeta_s + z_s;
                # the 0.8* product t1 doubles as the next step's mmE operand
                zeta = state.tile([80, N2], F32, tag="zeta")
                nc.gpsimd.tensor_add(zeta[:], t1_prev, z_dst)
                if s < L2 - 1:
                    t1 = state.tile([80, N2], F32, tag="t1")
                    nc.gpsimd.tensor_mul(t1[:], zeta[:], c08n[:])
                    # PE: finish bank_{s+1} = Act(IX) + wrec@(0.8*zeta_s) +
                    # wrec@z_s   (z-term last: it is the critical input)
                    nc.tensor.matmul(bank_n[:], wrec, t1_prev,
                                     start=False, stop=False,
                                     skip_group_check=True)
                    nc.tensor.matmul(bank_n[:], wrec, z_dst,
                                     start=False, stop=True,
                                     skip_group_check=True)
                    bank = bank_n
                    if s < L2 - 2:
                        bank_n = psB.tile([80, N2], F32, tag="bank")
                        nc.scalar.activation(bank_n[:], IXs[:, s + 2, :],
                                             ACTF.Copy, scale=1.0)
                    t1_prev = t1[:]
                vt_prev = vn[:]
                zeta_prev = zeta[:]
            # pass-end carries: gamma = C_L = zeta after the last step
            gamma = zeta_prev
            vt_end = vt_prev

        # ========== classifier: q = w_cls @ z + b_cls ======================
        Q16 = singles.tile([16, T], F32, tag="big_a")
        Zq = Z[0:81, :].rearrange("p (s c) -> p c s", c=N2)
        CPT = TT // L2  # chunks per t-tile
        for tt in range(NTT):
            ps = psA.tile([16, TT], F32, tag="mmps")
            nc.tensor.matmul(ps[:], wcls, Zq[:, CPT * tt:CPT * (tt + 1), :],
                             start=True, stop=True)
            dst = Q16[:, tt * TT:(tt + 1) * TT]
            nc.vector.tensor_copy(dst, ps[:])
        # repack [16, 8192] -> [128, 1024]: lane p = 16*g + (b*2+o)
        for g in range(8):
            nc.sync.dma_start(Q[16 * g:16 * (g + 1), :],
                              Q16[:, TL * g:TL * (g + 1)])

        # ================= LIF2: chunked 2-pass scan (128 lanes) ===========
        Qv = Q[:].rearrange("p (c s) -> p c s", s=L3)
        U3v = U3[:].rearrange("p (c s) -> p c s", s=L3)

        NH3 = N3 // 2
        prevs3 = []
        for h in range(2):
            u3 = state.tile([128, NH3], F32, tag=f"u3{h}")
            nc.vector.memset(u3[:], 0.0)
            prevs3.append(u3[:])
        for s in range(L3):
            gs = []
            for h in range(2):
                g = state.tile([128, NH3], F32, tag=f"g3{h}")
                nc.vector.scalar_tensor_tensor(out=g[:], in0=prevs3[h],
                                               scalar=VTH, in1=prevs3[h],
                                               op0=OP.is_le, op1=OP.mult)
                gs.append(g)
            for h in range(2):
                un = state.tile([128, NH3], F32, tag=f"u3{h}")
                nc.vector.scalar_tensor_tensor(
                    out=un[:], in0=gs[h][:], scalar=0.25,
                    in1=Qv[:, h * NH3:(h + 1) * NH3, s],
                    op0=OP.mult, op1=OP.add)
                prevs3[h] = un[:]
        uis3 = []
        for h in range(2):
            ui3 = state.tile([128, NH3], F32, tag=f"u3i{h}")
            if h == 0:
                nc.vector.memset(ui3[:, 0:1], 0.0)
                # carry across partition groups: lane p <- lane p-16 last chunk
                nc.sync.dma_start(ui3[16:128, 0:1],
                                  prevs3[1][0:112, NH3 - 1:NH3])
            else:
                nc.vector.tensor_copy(ui3[:, 0:1], prevs3[0][:, NH3 - 1:NH3])
            nc.vector.tensor_copy(ui3[:, 1:NH3], prevs3[h][:, 0:NH3 - 1])
            uis3.append(ui3[:])
        prevs3 = uis3
        for s in range(L3):
            gs = []
            for h in range(2):
                g = state.tile([128, NH3], F32, tag=f"g3{h}")
                nc.vector.scalar_tensor_tensor(out=g[:], in0=prevs3[h],
                                               scalar=VTH, in1=prevs3[h],
                                               op0=OP.is_le, op1=OP.mult)
                gs.append(g)
            for h in range(2):
                nc.vector.scalar_tensor_tensor(
                    out=U3v[:, h * NH3:(h + 1) * NH3, s], in0=gs[h][:],
                    scalar=0.25, in1=Qv[:, h * NH3:(h + 1) * NH3, s],
                    op0=OP.mult, op1=OP.add)
                prevs3[h] = U3v[:, h * NH3:(h + 1) * NH3, s]

        # ================= spike count + mean ==============================
        sp = singles.tile([128, TL], F32, tag="big_b")
        nc.vector.tensor_scalar(out=sp[:], in0=U3[:], scalar1=VTH,
                                scalar2=None, op0=OP.is_gt)
        red = small.tile([128, 1], F32, tag="red")
        nc.vector.tensor_reduce(out=red[:], in_=sp[:], axis=mybir.AxisListType.X,
                                op=OP.add)
        pso = psB.tile([16, 1], F32, tag="bank")
        nc.tensor.matmul(pso[:], ones_sum, red[:], start=True, stop=True)
        res = small.tile([16, 1], F32, tag="res")
        nc.scalar.activation(res[:], pso[:], ACTF.Copy, scale=1.0 / T)
        nc.sync.dma_start(out_d.ap(), res[:])


_NC_CACHE = None


def _get_program():
    global _NC_CACHE
    if _NC_CACHE is None:
        nc = bacc.Bacc("TRN2", target_bir_lowering=False, debug=False)
        emit_program(nc)
        nc.compile()
        _NC_CACHE = nc
    return _NC_CACHE


def make_in_maps(x, w_front, b_front, w_in, w_rec, w_cls, b_cls):
    x = np.asarray(x, np.float32)
    w_front = np.asarray(w_front, np.float32)
    b_front = np.asarray(b_front, np.float32)
    w_in = np.asarray(w_in, np.float32)
    w_rec = np.asarray(w_rec, np.float32)
    w_cls = np.asarray(w_cls, np.float32)
    b_cls = np.asarray(b_cls, np.float32)

    eeg = np.ascontiguousarray(x[:, 0, 1:-1, :])  # [B, C, T]

    wpack = np.zeros((128, 593), np.float32)
    for pair in range(4):
        for b2 in range(2):
            c0 = pair * 80 + pair * 20 + b2 * 10
            wpack[b2 * 64:(b2 + 1) * 64, c0:c0 + 10] = w_front.T
    wpack[0:80, 320] = np.tile(b_front, 8)
    for b in range(8):
        r = slice(b * 10, (b + 1) * 10)
        wpack[r, 321 + b * 10:321 + (b + 1) * 10] = w_in.T
        wpack[r, 401 + b * 10:401 + (b + 1) * 10] = w_rec.T
        wpack[r, 561 + b * 2:561 + (b + 1) * 2] = w_cls.T
        wpack[80, 561 + b * 2:561 + (b + 1) * 2] = b_cls
    for p in range(128):
        wpack[p, 577 + p % 16] = 1.0

    in_maps = []
    for c in range(NCORES):
        in_maps.append({
            "eeg": np.ascontiguousarray(eeg[c * BC:(c + 1) * BC]),
            "wpack": wpack,
        })
    return in_maps


def run_cores(in_maps, **kw):
    nc = _get_program()
    return run_bass_kernel_spmd(nc, in_maps, list(range(NCORES)), **kw)


def kernel(x, w_front, b_front, w_in, w_rec, w_cls, b_cls):
    in_maps = make_in_maps(x, w_front, b_front, w_in, w_rec, w_cls, b_cls)
    res = run_cores(in_maps)
    outs = [res.results[c]["out"].reshape(BC, O) for c in range(NCORES)]
    return np.concatenate(outs, axis=0).astype(np.float32)
